# revision 22
# baseline (speedup 1.0000x reference)
"""Trainium2 Bass kernel for the linear-attention block (nn_Attention).

Per batch element (x: [256, 4096] after flattening h*w):
    qkv = w_qkv @ x; q,k,v heads of 64
    q = softmax_d(q) * 64**-0.5 ; k = softmax_n(k) ; v = v/4096
    ctx[h] = k[h] @ v[h].T ; out[h] = ctx[h].T @ q[h]
    y = w_out @ out + b_out ; LayerNorm_c(y) * g

Sharding: data-parallel over batch, 2 batch elements per core, no collectives.

Host folds (exact): v/n into w_v; q-scale into w_out; LN mean-centering into
w_out columns (so on-chip LN only needs sum(y^2)); softmax max-subtraction
skipped (inputs ~N(0,1), exp safe).

Layout: k,v computed transposed ([n, feat]) directly by the matmul so the
ctx contraction over n is a plain PE accumulation and ksum is a ones-matmul;
q stays original ([feat, n]), its per-head column sums via block-diag ones
matmul, broadcast back via a second ones matmul; all big reciprocals are
reciprocal_approx_fast; ACT runs only Exp + batched Sqrt (table reloads cost
1.3us each); y^2 on GpSimd.
"""

import numpy as np

HEADS = 4
DIM_HEAD = 64
SCALE = DIM_HEAD ** -0.5
EPS = 1e-5
B, C, H, W = 16, 256, 64, 64
N = H * W  # 4096
HID = HEADS * DIM_HEAD  # 256
NCORES = 8
BPC = B // NCORES  # batches per core = 2

NT = N // 512    # 8 n-tiles of 512
NCH = N // 128   # 32 n-chunks of 128
CT = C // 128    # 2 contraction tiles

_cache = {}


def _build_nc():
    import concourse.bass as bass
    import concourse.tile as tile
    from concourse import bacc, mybir

    f32 = mybir.dt.float32
    bf16 = mybir.dt.bfloat16
    AF = mybir.ActivationFunctionType
    OP = mybir.AluOpType

    nc = bacc.Bacc(None, target_bir_lowering=False, debug=False)
    x_ext = nc.declare_dram_parameter("x", [BPC, C, N], bf16, isOutput=False)
    wqkvT_ext = nc.declare_dram_parameter("wqkvT", [C, 3 * HID], bf16, isOutput=False)
    woutcT_ext = nc.declare_dram_parameter("woutcT", [HID, C], bf16, isOutput=False)
    bc_ext = nc.declare_dram_parameter("bc", [C, 1], f32, isOutput=False)
    g_ext = nc.declare_dram_parameter("g", [C, 1], f32, isOutput=False)
    onesbc_ext = nc.declare_dram_parameter("onesbc", [2, 128], bf16, isOutput=False)
    out_ext = nc.declare_dram_parameter("out", [BPC, C, N], f32, isOutput=True)

    with tile.TileContext(nc) as tc:
        with (
            tc.tile_pool(name="wts", bufs=1) as wts,
            tc.tile_pool(name="xs", bufs=2) as xs_pool,
            tc.tile_pool(name="qexp", bufs=2) as qexp_pool,
            tc.tile_pool(name="kv", bufs=4) as kv_pool,
            tc.tile_pool(name="small", bufs=4) as small_pool,
            tc.tile_pool(name="mid", bufs=4) as mid_pool,
            tc.tile_pool(name="fin", bufs=4) as fin_pool,
            tc.tile_pool(name="psum_mm", bufs=6, space="PSUM") as pmm,
            tc.tile_pool(name="psum_acc", bufs=1, space="PSUM") as pacc,
        ):
            # ---- constants & weights (loaded once, one DMA per tensor) ----
            wqkvT3 = wts.tile([128, CT, 3 * HID], bf16, tag="wqkvT", name="wqkvT")
            nc.sync.dma_start(out=wqkvT3, in_=wqkvT_ext[:, :].rearrange("(i p) o -> p i o", p=128))
            wqkvT = [wqkvT3[:, i] for i in range(CT)]
            woutcT3 = wts.tile([128, CT, C], bf16, tag="woutcTb", name="woutcTb")
            nc.sync.dma_start(out=woutcT3, in_=woutcT_ext[:, :].rearrange("(i p) o -> p i o", p=128))
            woutcT = [woutcT3[:, i] for i in range(CT)]
            bcg = wts.tile([128, 2, CT], f32, tag="bcg", name="bcg")
            nc.sync.dma_start(out=bcg[:, 0], in_=bc_ext[:, :].rearrange("(i p) o -> p (i o)", p=128))
            nc.sync.dma_start(out=bcg[:, 1], in_=g_ext[:, :].rearrange("(i p) o -> p (i o)", p=128))
            bc_sb = [bcg[:, 0, i:i + 1] for i in range(CT)]
            g_sb = [bcg[:, 1, i:i + 1] for i in range(CT)]

            ones128 = wts.tile([128, 1], bf16, tag="ones128", name="ones128")
            nc.vector.memset(ones128, 1.0)
            onesblk = wts.tile([128, 2], bf16, tag="onesblk", name="onesblk")
            nc.vector.memset(onesblk, 0.0)
            nc.vector.memset(onesblk[0:64, 0:1], 1.0)
            nc.vector.memset(onesblk[64:128, 1:2], 1.0)
            onesbc = wts.tile([2, 128], bf16, tag="onesbc", name="onesbc")
            nc.sync.dma_start(out=onesbc, in_=onesbc_ext[:, :])
            ones1 = wts.tile([1, 128], bf16, tag="ones1", name="ones1")
            nc.vector.memset(ones1, 1.0)
            eps_sb = wts.tile([128, 1], f32, tag="eps", name="eps")
            nc.vector.memset(eps_sb, EPS)

            for b in range(BPC):
                # ---- load x ----
                xs3 = xs_pool.tile([128, CT, N], bf16, tag="x", name="x")
                nc.sync.dma_start(out=xs3, in_=x_ext[b].rearrange("(i p) n -> p i n", p=128))
                xs = [xs3[:, i] for i in range(CT)]

                # ---- stage A: kT/vT chunks + ksum + ctx accumulation ----
                ksum_t = pacc.tile([128, 2], f32, tag="ksum", name="ksum")
                ctx_t = pacc.tile([128, 256], f32, tag="ctx", name="ctx")
                ksum_ps = [ksum_t[:, i:i + 1] for i in range(CT)]
                ctx_ps = [ctx_t[:, i * 128:(i + 1) * 128] for i in range(2)]
                for nch in range(NCH):
                    kv_ps = pmm.tile([128, 512], f32, tag="mm", name="mm")
                    for ct in range(CT):
                        nc.tensor.matmul(
                            kv_ps,
                            xs[ct][:, nch * 128:(nch + 1) * 128],
                            wqkvT[ct][:, HID:3 * HID],
                            start=(ct == 0), stop=(ct == CT - 1),
                        )
                    kexp_t = kv_pool.tile([128, HID], bf16, tag="kexp", name="kexp")
                    nc.scalar.activation(out=kexp_t, in_=kv_ps[:, 0:HID], func=AF.Exp)
                    v_t = kv_pool.tile([128, HID], bf16, tag="v", name="v")
                    nc.scalar.copy(out=v_t, in_=kv_ps[:, HID:2 * HID])
                    for i in range(CT):
                        # one bank holds both ksum chains: only the very first
                        # matmul clears has_written; later groups rely on the
                        # per-element has_written bits
                        nc.tensor.matmul(
                            ksum_ps[i],
                            kexp_t[:, i * 128:(i + 1) * 128],
                            ones128,
                            start=(nch == 0 and i == 0),
                            stop=(nch == NCH - 1 and i == CT - 1),
                            skip_group_check=True,
                        )
                    for hp in range(2):
                        nc.tensor.matmul(
                            ctx_ps[hp],
                            kexp_t[:, hp * 128:(hp + 1) * 128],
                            v_t[:, hp * 128:(hp + 1) * 128],
                            start=(nch == 0 and hp == 0),
                            stop=(nch == NCH - 1 and hp == 1),
                            skip_group_check=True,
                        )

                # ---- stage B: normalize ctx rows by 1/ksum (block-diag layout) ----
                krecip = [small_pool.tile([128, 1], f32, tag=f"krecip{i}", name=f"krecip{i}") for i in range(CT)]
                for i in range(CT):
                    nc.vector.reciprocal(out=krecip[i], in_=ksum_ps[i])
                ctx_sb = [small_pool.tile([128, 128], bf16, tag=f"ctxsb{i}", name=f"ctxsb{i}") for i in range(2)]
                for hp in range(2):
                    nc.vector.memset(ctx_sb[hp], 0.0)
                    for hh in range(2):
                        s = slice(hh * 64, hh * 64 + 64)
                        nc.vector.tensor_scalar(
                            out=ctx_sb[hp][s, s],
                            in0=ctx_ps[hp][s, s],
                            scalar1=krecip[hp][s],
                            scalar2=None,
                            op0=OP.mult,
                        )

                # ---- stage C: q (original layout), out = ctx^T @ q, y matmul ----
                qexp = [qexp_pool.tile([128, N], bf16, tag=f"qexp{i}", name=f"qexp{i}") for i in range(2)]
                y_sb = [fin_pool.tile([128, N], f32, tag=f"y{i}", name=f"y{i}", bufs=1) for i in range(2)]
                y2_sb = [fin_pool.tile([128, N], bf16, tag=f"y2{i}", name=f"y2{i}", bufs=1) for i in range(2)]
                for nt in range(NT):
                    nsl = slice(nt * 512, (nt + 1) * 512)
                    out_tiles = []
                    for qt in range(2):
                        q_ps = pmm.tile([128, 512], f32, tag="mm", name="mm")
                        for ct in range(CT):
                            nc.tensor.matmul(
                                q_ps,
                                wqkvT[ct][:, qt * 128:(qt + 1) * 128],
                                xs[ct][:, nsl],
                                start=(ct == 0), stop=(ct == CT - 1),
                            )
                        nc.scalar.activation(out=qexp[qt][:, nsl], in_=q_ps, func=AF.Exp)
                        qs_ps = pmm.tile([2, 512], f32, tag="mm", name="mm")
                        nc.tensor.matmul(qs_ps, onesblk, qexp[qt][:, nsl],
                                         start=True, stop=True)
                        qs_sb = small_pool.tile([2, 512], bf16, tag="qs", name="qs")
                        nc.vector.tensor_copy(out=qs_sb, in_=qs_ps)
                        qbc_ps = pmm.tile([128, 512], f32, tag="mm", name="mm")
                        nc.tensor.matmul(qbc_ps, onesbc, qs_sb, start=True, stop=True)
                        qbcr = mid_pool.tile([128, 512], f32, tag="qbcr", name="qbcr")
                        nc.vector.reciprocal_approx_fast(out=qbcr, in_=qbc_ps)
                        o_ps = pmm.tile([128, 512], f32, tag="mm", name="mm")
                        nc.tensor.matmul(o_ps, ctx_sb[qt], qexp[qt][:, nsl],
                                         start=True, stop=True)
                        out_sb = mid_pool.tile([128, 512], bf16, tag="outsb", name="outsb")
                        nc.vector.tensor_mul(out=out_sb, in0=o_ps, in1=qbcr)
                        out_tiles.append(out_sb)
                    for ot in range(2):
                        y_ps = pmm.tile([128, 512], f32, tag="mm", name="mm")
                        for et in range(2):
                            nc.tensor.matmul(
                                y_ps,
                                woutcT[et][:, ot * 128:(ot + 1) * 128],
                                out_tiles[et],
                                start=(et == 0), stop=(et == 1),
                            )
                        nc.vector.tensor_scalar_add(
                            out=y_sb[ot][:, nsl], in0=y_ps, scalar1=bc_sb[ot])
                        nc.gpsimd.tensor_mul(
                            out=y2_sb[ot][:, nsl], in0=y_sb[ot][:, nsl],
                            in1=y_sb[ot][:, nsl])
                # ---- stage D: LN epilogue (sqrts batched to limit ACT table loads) ----
                s2_sb = small_pool.tile([1, NT, 512], bf16, tag="s2", name="s2")
                sq_list = []
                for nt in range(NT):
                    nsl = slice(nt * 512, (nt + 1) * 512)
                    s2_ps = pmm.tile([1, 512], f32, tag="mm", name="mm")
                    for ot in range(2):
                        nc.tensor.matmul(s2_ps, ones128, y2_sb[ot][:, nsl],
                                         start=(ot == 0), stop=(ot == 1))
                    nc.vector.tensor_copy(out=s2_sb[:, nt], in_=s2_ps)
                for nt in range(NT):
                    s2bc_ps = pmm.tile([128, 512], f32, tag="mm", name="mm")
                    nc.tensor.matmul(s2bc_ps, ones1, s2_sb[:, nt], start=True, stop=True)
                    sq_sb = mid_pool.tile([128, 512], f32, tag="sq", name="sq")
                    nc.scalar.activation(out=sq_sb, in_=s2bc_ps, func=AF.Sqrt,
                                         bias=eps_sb, scale=1.0 / C)
                    sq_list.append(sq_sb)
                for nt in range(NT):
                    nsl = slice(nt * 512, (nt + 1) * 512)
                    rstd = mid_pool.tile([128, 512], f32, tag="rstd", name="rstd")
                    nc.vector.reciprocal_approx_fast(out=rstd, in_=sq_list[nt])
                    for ot in range(2):
                        fin = fin_pool.tile([128, 512], f32, tag="fin", name="fin")
                        nc.vector.scalar_tensor_tensor(
                            out=fin,
                            in0=y_sb[ot][:, nsl],
                            scalar=g_sb[ot],
                            in1=rstd,
                            op0=OP.mult,
                            op1=OP.mult,
                        )
                        nc.sync.dma_start(
                            out=out_ext[b, ot * 128:(ot + 1) * 128, nt * 512:(nt + 1) * 512], in_=fin
                        )
    nc.compile()
    return nc


def _prep_weights(w_qkv, w_out, b_out):
    import ml_dtypes
    w_qkv = np.asarray(w_qkv, dtype=np.float64)
    w_out = np.asarray(w_out, dtype=np.float64)
    b_out = np.asarray(b_out, dtype=np.float64)
    wq = w_qkv.copy()
    wq[2 * HID:3 * HID, :] /= N          # fold v/n
    wqkvT = np.ascontiguousarray(wq.T).astype(ml_dtypes.bfloat16)
    wo = w_out * SCALE                    # fold q scale
    wo = wo - wo.mean(axis=0, keepdims=True)  # fold LN mean-centering
    woutcT = np.ascontiguousarray(wo.T).astype(ml_dtypes.bfloat16)
    bc = (b_out - b_out.mean()).astype(np.float32).reshape(C, 1)
    return wqkvT, woutcT, bc


def kernel(x, w_qkv, w_out, b_out, g):
    import ml_dtypes
    from concourse.bass_utils import run_bass_kernel_spmd

    if "nc" not in _cache:
        _cache["nc"] = _build_nc()
    nc = _cache["nc"]

    xf = np.ascontiguousarray(np.asarray(x, dtype=np.float32).reshape(B, C, N).astype(ml_dtypes.bfloat16))
    wqkvT, woutcT, bc = _prep_weights(w_qkv, w_out, b_out)
    g2 = np.asarray(g, dtype=np.float32).reshape(C, 1)

    in_maps = []
    for i in range(NCORES):
        onesbc = np.zeros((2, 128), dtype=ml_dtypes.bfloat16)
        onesbc[0, 0:64] = 1.0
        onesbc[1, 64:128] = 1.0
        in_maps.append({
            "x": np.ascontiguousarray(xf[i * BPC:(i + 1) * BPC]),
            "wqkvT": wqkvT,
            "woutcT": woutcT,
            "bc": bc,
            "g": g2,
            "onesbc": onesbc,
        })
    res = run_bass_kernel_spmd(nc, in_maps, core_ids=list(range(NCORES)))
    outs = [res.results[i]["out"] for i in range(NCORES)]
    y = np.concatenate(outs, axis=0).reshape(B, C, H, W).astype(np.float32)
    return y
